# revision 48
# baseline (speedup 1.0000x reference)
"""Tensor-parallel attention kernel for Trainium2 (8 NeuronCores).

Problem: B=1, L=2048, D=4096, H=32 q-heads, KV=8 kv-heads, HD=128,
partial rotary ROT=64, causal additive mask, o-projection.

Sharding: TP-8 over heads. Core c owns q-heads 4c..4c+3 and kv-head c
(column shard of w_qkv), plus the matching row shard of w_o. Each core
computes a full [L, D] partial of the output; the host sums the 8
partials (the cross-core reduction of the row-sharded o-projection).

Precision plan: fp16 operands everywhere (PSUM accumulation fp32) —
measured ~216 ns per N=512 matmul vs ~272 ns for fp32r, and half the
HBM traffic. (fp8 attention was tried and rejected: softmax
probabilities span ~e^21 of dynamic range, which neither e4m3 nor
e5m2 can represent within the 2e-2 error budget.) The exp is shifted
(exp(s-5)); the shift cancels in the normalization. Host pre-arranges
x, w_qkv, w_o and the rope tables partition-major so every DMA moves
large contiguous per-partition lines, with w_qkv in ct-major slabs so
the first accumulation group only gates on 1 MB.

On-chip orientation: everything transposed so matmuls contract over
the partition dim with no activation transposes:
  qkvT[col, L] = w_qkv.T @ x.T          (w stationary, xT streamed)
  rope:  qT' = qT * cosE + (P @ qT) * sinE   (P = rotate-half matrix on PE)
  ST[k, q]   = kT_tile.T @ qT            (one matmul per k-tile, K=HD=128)
  PT         = exp(ST - 5)  (diagonal tiles then get a 0/1 visibility mul)
  den[*, q]  = ones.T @ PT               (ones-matmul, accumulated over k)
  oT[d, q]   = V_tile.T @ PT             (V from a one-time PE transpose of vT)
  out[l, e]  = (oT/den).T @ w_o_shard    (partial; summed across cores on host)
"""

import sys

for _p in ("/opt/trn_rl_repo", "/root/.axon_site/_ro/trn_rl_repo"):
    if _p not in sys.path:
        sys.path.append(_p)

import numpy as np

B, L, D = 1, 2048, 4096
H, KV, HD = 32, 8, 128
ROT = 64
SCALE = HD ** -0.5
NEG = -1e9
NCORES = 8
HPC = H // NCORES          # q-heads per core (4)
CPC = HPC * HD + 2 * HD    # w_qkv columns per core (768)
NDT = D // 128             # contraction tiles over D (32)
NKT = L // 128             # k tiles (16)
NJQ = L // 512             # 512-wide q blocks (4)
XBLK = 512                 # L-block width in the qkv phase
NLB = L // XBLK            # 4
EXPSHIFT = -5.0            # softmax exp shift; cancels in normalization
MASKNEG = -30000.0         # additive mask for diagonal tiles (fp16-safe)

_cache = {}


def _build(causal: bool):
    import concourse.mybir as mybir
    import concourse.tile as tile
    from concourse import bacc

    F32 = mybir.dt.float32
    F16 = mybir.dt.float16
    EXP = mybir.ActivationFunctionType.Exp

    nc = bacc.Bacc("TRN2", target_bir_lowering=False, debug=False)

    xt = nc.dram_tensor("xt", [NLB, 128, NDT, XBLK], F16, kind="ExternalInput").ap()
    # ct-major: [6, 128, NDT, 128] so the first matmul group only needs the
    # first 1 MB slab, not all 6.3 MB
    wqkv = nc.dram_tensor("wqkv", [6, 128, NDT, 128], F16, kind="ExternalInput").ap()
    wo = nc.dram_tensor("wo", [128, HPC, D], F16, kind="ExternalInput").ap()
    cos_e = nc.dram_tensor("cos_e", [NLB, 128, 2, XBLK], F16, kind="ExternalInput").ap()
    sin_e = nc.dram_tensor("sin_e", [NLB, 128, 2, XBLK], F16, kind="ExternalInput").ap()
    consts = nc.dram_tensor("consts", [128, 384], F16, kind="ExternalInput").ap()
    if causal:
        # additive mask for the 4 diagonal k-tile offsets: 0 / -30000
        dmadd = nc.dram_tensor("dmadd", [128, 4, 512], F16,
                               kind="ExternalInput").ap()
    else:
        mask_t = nc.dram_tensor("mask_t", [L, L], F16, kind="ExternalInput").ap()
    out_p = nc.dram_tensor("out_p", [L, D], F16, kind="ExternalOutput").ap()

    with tile.TileContext(nc) as tc:
        with tc.tile_pool(name="persist", bufs=1) as persist:
            kt_sb = persist.tile([128, L], F16, tag="kt")
            v_sb = persist.tile([128, NKT, 128], F16, tag="v")
            qt_sb = persist.tile([128, HPC, L], F16, tag="qt")
            otn_sb = persist.tile([128, HPC, L], F16, tag="otn")
            cst = persist.tile([128, 384], F16, tag="cst")
            dm_sb = persist.tile([128, 4, 512], F16, tag="dm")
            expb = persist.tile([128, 1], F32, tag="expb")
            nc.gpsimd.memset(expb, EXPSHIFT)
            nc.sync.dma_start(out=cst, in_=consts)
            ident = cst[:, 0:128]
            pmat_t = cst[:, 128:256]
            ones = cst[:, 256:384]

            # ---------------- Phase 1: qkv projection + rope ----------------
            with tc.tile_pool(name="wq", bufs=1) as wqp, \
                 tc.tile_pool(name="xb", bufs=2) as xbp, \
                 tc.tile_pool(name="tabs", bufs=2) as tabs, \
                 tc.tile_pool(name="stage", bufs=3) as stage, \
                 tc.tile_pool(name="vtmp", bufs=2) as vtmp, \
                 tc.tile_pool(name="ps1", bufs=6, space="PSUM") as ps1, \
                 tc.tile_pool(name="psr", bufs=1, space="PSUM") as psr:
                wq_sb = wqp.tile([128, 6, NDT, 128], F16)

                # PE warm-up while the first slabs stream in: gets the HAM
                # clock gate to 8/8 before the real matmuls begin
                wps = psr.tile([128, 384], F32, tag="rot", name="warmup",
                               bufs=2)
                for _ in range(24):
                    nc.tensor.matmul(out=wps, lhsT=ident, rhs=cst,
                                     start=True, stop=True)

                # deferred tail-work (PE rot matmul / v transposes) per (lb,
                # ct), emitted one-to-two matmul-groups later so the PE never
                # stalls waiting on the ACT copy of a group's PSUM.
                pending = []

                def flush_pending(n=99):
                    while pending and n > 0:
                        pending.pop(0)()
                        n -= 1

                def post_group(lb, ct, acc, cosb, sinb):
                    if ct == 5:
                        vt = vtmp.tile([128, XBLK], F16, tag="vt",
                                       name=f"vt_{lb}")
                        nc.scalar.copy(out=vt, in_=acc)

                        def fin_v(lb=lb, vt=vt):
                            # DMA-engine transpose: vT [128d, 512l] ->
                            # v [4x128 l-rows, 128 d], keeping the PE free
                            kk = XBLK // 128
                            nc.sync.dma_start_transpose(
                                out=v_sb[:, kk * lb:kk * (lb + 1), :], in_=vt)

                        pending.append(fin_v)
                        return
                    # rope for q (ct 0..3, scaled tables) and k (ct 4)
                    ti = 0 if ct < 4 else 1
                    s_sb = stage.tile([128, XBLK], F16, tag="s_sb",
                                      name=f"s_sb_{lb}_{ct}", bufs=6)
                    nc.scalar.copy(out=s_sb, in_=acc)

                    def fin_rope(ct=ct, s_sb=s_sb, cosb=cosb, sinb=sinb,
                                 ti=ti, lb=lb):
                        ls = slice(lb * XBLK, (lb + 1) * XBLK)
                        # rotate-half via DVE quadrant shuffles instead of a
                        # PE matmul; the minus sign of [-x2, x1] is folded
                        # into the host sin table (rows 0:32 negated). Rows
                        # >= 64 need no rotation (sin rows there are 0).
                        rsh = stage.tile([64, XBLK], F16, tag="rsh",
                                         name=f"rsh_{lb}_{ct}", bufs=2)
                        idm = list(range(32))
                        nc.vector.stream_shuffle(rsh[0:32, :], s_sb[32:64, :], idm)
                        nc.vector.stream_shuffle(rsh[32:64, :], s_sb[0:32, :], idm)
                        dst = kt_sb[:, ls] if ct == 4 else qt_sb[:, ct, ls]
                        nc.vector.tensor_mul(dst, s_sb, cosb[:, ti, :])
                        m2 = stage.tile([64, XBLK], F16, tag="m2",
                                        name=f"m2_{lb}_{ct}")
                        nc.vector.tensor_mul(m2, rsh, sinb[0:64, ti, :])
                        dst64 = kt_sb[0:64, ls] if ct == 4 \
                            else qt_sb[0:64, ct, ls]
                        nc.vector.tensor_add(dst64, dst64, m2)

                    pending.append(fin_rope)

                for lb in range(NLB):
                    xblk = xbp.tile([128, NDT, XBLK], F16, tag="xblk")
                    cosb = tabs.tile([128, 2, XBLK], F16, tag="cosb")
                    sinb = tabs.tile([128, 2, XBLK], F16, tag="sinb")
                    if lb == 0:
                        # first block: stream x chunks and wqkv slab pieces
                        # in consumption order across the 3 DMA queues, and
                        # interleave the 6 accumulations dti-outer so the PE
                        # consumes each chunk 6 times as it arrives
                        for cg in range(4):
                            cgs = slice(cg * 8, cg * 8 + 8)
                            nc.gpsimd.dma_start(out=xblk[:, cg * 8:cg * 8 + 4, :],
                                                in_=xt[lb, :, cg * 8:cg * 8 + 4, :])
                            nc.scalar.dma_start(out=xblk[:, cg * 8 + 4:cg * 8 + 8, :],
                                                in_=xt[lb, :, cg * 8 + 4:cg * 8 + 8, :])
                            for ct in range(6):
                                weng = (nc.sync, nc.sync, nc.sync, nc.sync,
                                        nc.gpsimd, nc.scalar)[ct]
                                weng.dma_start(out=wq_sb[:, ct, cgs, :],
                                               in_=wqkv[ct, :, cgs, :])
                        # rope tables aren't read until the first rope finish
                        # (~25us in); keep them behind the critical x chunks
                        nc.scalar.dma_start(out=cosb, in_=cos_e[lb])
                        nc.scalar.dma_start(out=sinb, in_=sin_e[lb])
                        accs0 = [ps1.tile([128, XBLK], F32, tag="acc",
                                          name=f"acc0_{ct}") for ct in range(6)]
                        for cg in range(4):
                            for ct in range(6):
                                for dti in range(cg * 8, cg * 8 + 8):
                                    nc.tensor.matmul(
                                        out=accs0[ct],
                                        lhsT=wq_sb[:, ct, dti, :],
                                        rhs=xblk[:, dti, :],
                                        start=(dti == 0), stop=(dti == NDT - 1))
                        for ct in range(6):
                            post_group(lb, ct, accs0[ct], cosb, sinb)
                        continue
                    # chunked so block-1 matmuls can start before the whole
                    # 4 MB block has landed (blocks 2-3 are prefetched anyway);
                    # block 1 splits across two queues since it races block-0
                    # traffic
                    for cg in range(4):
                        xeng = nc.scalar if (lb == 1 and cg % 2 == 1) else nc.gpsimd
                        xeng.dma_start(out=xblk[:, cg * 8:cg * 8 + 8, :],
                                       in_=xt[lb, :, cg * 8:cg * 8 + 8, :])
                    nc.sync.dma_start(out=cosb, in_=cos_e[lb])
                    nc.sync.dma_start(out=sinb, in_=sin_e[lb])
                    if lb == 1 and causal:
                        # needed from phase 2 on; off the hot queues
                        nc.scalar.dma_start(out=dm_sb, in_=dmadd)
                    for ct in range(6):
                        acc = ps1.tile([128, XBLK], F32, tag="acc",
                                       name=f"acc_{lb}_{ct}")
                        for dti in range(NDT):
                            nc.tensor.matmul(
                                out=acc,
                                lhsT=wq_sb[:, ct, dti, :],
                                rhs=xblk[:, dti, :],
                                start=(dti == 0), stop=(dti == NDT - 1))
                        # drain faster in the last block so the rope tail
                        # doesn't delay the phase-2 PSUM handoff
                        flush_pending(3 if lb == NLB - 1 else 2)
                        post_group(lb, ct, acc, cosb, sinb)
                flush_pending()

            # wo shard stays resident through phases 2+3
            with tc.tile_pool(name="wop", bufs=1) as wop:
                wo_sb = wop.tile([128, HPC, D], F16)
                nc.gpsimd.dma_start(out=wo_sb, in_=wo)

                # ---------------- Phase 2: attention ----------------
                with tc.tile_pool(name="ptp", bufs=4) as ptp, \
                     tc.tile_pool(name="mb", bufs=2) as mbp, \
                     tc.tile_pool(name="rdp", bufs=2) as rdp, \
                     tc.tile_pool(name="ps_st", bufs=2, space="PSUM") as ps_st, \
                     tc.tile_pool(name="ps_acc", bufs=2, space="PSUM") as ps_acc:
                    for jq in range(NJQ):
                        qs = slice(jq * 512, (jq + 1) * 512)
                        nkt = 4 * (jq + 1) if causal else NKT
                        diag0 = 4 * jq
                        if not causal:
                            mblk = mbp.tile([128, NKT, 512], F16, tag="mblk")
                            nc.sync.dma_start(
                                out=mblk,
                                in_=mask_t[:, qs].rearrange("(kt p) q -> p kt q", p=128))
                        npair = nkt // 2
                        # diagonal pairs first: their DVE mask-muls then
                        # overlap later pairs' matmuls instead of sitting on
                        # the exp critical path at the iteration tail
                        if causal and npair >= 2:
                            order = [npair - 2, npair - 1] + list(range(npair - 2))
                        else:
                            order = list(range(npair))

                        # flat (h, pair) stream: the 2-deep lookahead carries
                        # across head boundaries so the PE never drains
                        hctx = {}
                        for h in range(HPC):
                            hctx[h] = {
                                "den": ps_acc.tile([128, 512], F32, tag="den",
                                                   name=f"den_{jq}_{h}"),
                                "ot": ps_acc.tile([128, 512], F32, tag="ot",
                                                  name=f"ot_{jq}_{h}"),
                                "n": 0,
                            }

                        def tile_off(i, diag0=diag0):
                            # visible region of k-tile i is q >= 128*o for
                            # diagonal offset o; off-diagonal tiles are full
                            if not causal or i < diag0:
                                return 0
                            return 128 * (i - diag0)

                        def emit_den_ot(h, g, pts, jq=jq, diag0=diag0,
                                        npair=npair):
                            c = hctx[h]
                            pt = pts.pop((h, g))
                            if causal:
                                # 0/1 visibility applied to the fp16
                                # probabilities, off the exp critical path
                                for t in (0, 1):
                                    i = 2 * g + t
                                    if i >= diag0:
                                        o = i - diag0
                                        q0 = 128 * o
                                        nc.vector.tensor_mul(
                                            pt[:, t, q0:], pt[:, t, q0:],
                                            dm_sb[:, o, q0:])
                            for t in (0, 1):
                                i = 2 * g + t
                                q0 = tile_off(i)
                                first = c["n"] == 0
                                last = c["n"] == 2 * npair - 1
                                # the first emitted matmul is always the
                                # full-width diagonal o=0 tile, so start=True
                                # initializes the whole PSUM row
                                nc.tensor.matmul(
                                    out=c["den"][:, q0:], lhsT=ones,
                                    rhs=pt[:, t, q0:],
                                    start=first, stop=last)
                                nc.tensor.matmul(
                                    out=c["ot"][:, q0:], lhsT=v_sb[:, i, :],
                                    rhs=pt[:, t, q0:],
                                    start=first, stop=last)
                                c["n"] += 1
                            if last:
                                rd = rdp.tile([128, 512], F32, tag="rd",
                                              name=f"rd_{jq}_{h}")
                                nc.vector.reciprocal_approx_fast(
                                    out=rd, in_=c["den"])
                                nc.vector.tensor_mul(
                                    otn_sb[:, h, qs], c["ot"], rd)

                        pts = {}
                        inflight = []
                        for h in range(HPC):
                            for g in order:
                                st = ps_st.tile([128, 2, 512], F32, tag="st",
                                                name=f"st_{jq}_{h}_{g}")
                                for t in (0, 1):
                                    i = 2 * g + t
                                    q0 = tile_off(i)
                                    nc.tensor.matmul(
                                        out=st[:, t, q0:],
                                        lhsT=kt_sb[:, i * 128:(i + 1) * 128],
                                        rhs=qt_sb[:, h, jq * 512 + q0:(jq + 1) * 512],
                                        start=True, stop=True)
                                if not causal:
                                    for t in (0, 1):
                                        nc.vector.tensor_add(
                                            st[:, t, :], st[:, t, :],
                                            mblk[:, 2 * g + t, :])
                                pt = ptp.tile([128, 2, 512], F16, tag="pt",
                                              name=f"pt_{jq}_{h}_{g}")
                                nc.scalar.activation(pt, st, EXP, bias=expb)
                                pts[(h, g)] = pt
                                inflight.append((h, g))
                                if len(inflight) > 2:
                                    emit_den_ot(*inflight.pop(0), pts)
                        while inflight:
                            emit_den_ot(*inflight.pop(0), pts)

                # ---------------- Phase 3: o-projection ----------------
                with tc.tile_pool(name="ost", bufs=2) as ostp, \
                     tc.tile_pool(name="ps3", bufs=8, space="PSUM") as ps3:
                    for lt in range(L // 128):
                        lsl = slice(lt * 128, (lt + 1) * 128)
                        ostage = ostp.tile([128, D // 512, 512], F16, tag="ostage")
                        for eg in range(2):
                            accs = [ps3.tile([128, 512], F32, tag="acc3",
                                             name=f"acc3_{lt}_{eg}_{k}")
                                    for k in range(4)]
                            for h in range(HPC):
                                for e4 in range(4):
                                    et = eg * 4 + e4
                                    nc.tensor.matmul(
                                        out=accs[e4],
                                        lhsT=otn_sb[:, h, lsl],
                                        rhs=wo_sb[:, h, et * 512:(et + 1) * 512],
                                        start=(h == 0), stop=(h == HPC - 1))
                            for e4 in range(4):
                                et = eg * 4 + e4
                                if e4 % 2 == 0:
                                    nc.vector.tensor_copy(ostage[:, et, :], accs[e4])
                                else:
                                    nc.scalar.copy(out=ostage[:, et, :], in_=accs[e4])
                            nc.sync.dma_start(
                                out=out_p[lsl, eg * 2048:(eg + 1) * 2048],
                                in_=ostage[:, eg * 4:(eg + 1) * 4, :])

    nc.compile()
    return nc


def _host_inputs(x, attention_mask, cos, sin, w_qkv, w_o, causal):
    """Build the 8 per-core input maps (fp16, partition-major)."""
    F16 = np.float16

    # x.T pre-tiled: [NLB, 128, NDT, XBLK], contiguous per partition
    xT = np.ascontiguousarray(x[0].T)                     # [D, L]
    xt_h = np.ascontiguousarray(
        xT.reshape(NDT, 128, NLB, XBLK).transpose(2, 1, 0, 3)).astype(F16)
    q_pos = H * HD
    kv_pos = q_pos + KV * HD

    # extended rope tables: slot 0 = q (scale folded), slot 1 = k
    # row d<64: cos[l, d]; row d>=64: 1.0 (cos) / 0.0 (sin)
    cos_t = cos.T.astype(np.float32)                      # [ROT, L]
    sin_t = sin.T.astype(np.float32)
    cos_e = np.empty((2, 128, L), np.float32)
    sin_e = np.zeros((2, 128, L), np.float32)
    cos_e[0, :ROT] = cos_t * SCALE
    cos_e[0, ROT:] = SCALE
    cos_e[1, :ROT] = cos_t
    cos_e[1, ROT:] = 1.0
    sin_e[0, :ROT] = sin_t * SCALE
    sin_e[1, :ROT] = sin_t
    # rotate-half sign ([-x2, x1]) folded into the table: the kernel's DVE
    # shuffle produces the plain swap [x2, x1]
    sin_e[:, :32] *= -1.0
    # -> [NLB, 128, 2, XBLK]
    cs_h = np.ascontiguousarray(
        cos_e.reshape(2, 128, NLB, XBLK).transpose(2, 1, 0, 3)).astype(F16)
    sn_h = np.ascontiguousarray(
        sin_e.reshape(2, 128, NLB, XBLK).transpose(2, 1, 0, 3)).astype(F16)

    # consts [128, 384] = [identity | pmat_t | ones]
    pmat = np.zeros((128, 128), np.float32)
    for dp in range(32):
        pmat[dp, dp + 32] = -1.0
    for dp in range(32, 64):
        pmat[dp, dp - 32] = 1.0
    consts = np.concatenate(
        [np.eye(128, dtype=np.float32), pmat.T,
         np.ones((128, 128), np.float32)], axis=1).astype(F16)

    mask2d = np.ascontiguousarray(attention_mask[0, 0])   # [L(q), L(k)]
    if causal:
        mask_t_full = None
        # 0/1 visibility for diagonal k-tiles: offset o -> visible iff
        # q_local >= k_local + 128*o  (pt layout is [k, o, q])
        kloc = np.arange(128)[:, None]
        qloc = np.arange(512)[None, :]
        dmadd = np.empty((128, 4, 512), np.float32)
        for o in range(4):
            dmadd[:, o, :] = (qloc >= kloc + 128 * o)
        dmadd = np.ascontiguousarray(dmadd.astype(F16))
    else:
        mask_t_full = np.ascontiguousarray(
            np.maximum(mask2d.T, MASKNEG)).astype(F16)    # [k, q]
        dmadd = None

    in_maps = []
    for c in range(NCORES):
        cols = []
        for j in range(HPC):
            h = c * HPC + j
            cols.append(w_qkv[:, h * HD:(h + 1) * HD])
        cols.append(w_qkv[:, q_pos + c * HD:q_pos + (c + 1) * HD])
        cols.append(w_qkv[:, kv_pos + c * HD:kv_pos + (c + 1) * HD])
        wqkv_c = np.concatenate(cols, axis=1)             # [D, 768]
        # ct-major slabs: [6, 128, NDT, 128]
        wqkv_h = np.ascontiguousarray(
            wqkv_c.reshape(NDT, 128, 6, 128).transpose(2, 1, 0, 3)).astype(F16)
        wo_c = w_o[c * HPC * HD:(c + 1) * HPC * HD, :]    # [512, D]
        wo_h = np.ascontiguousarray(
            wo_c.reshape(HPC, 128, D).transpose(1, 0, 2)).astype(F16)
        m = {"xt": xt_h, "wqkv": wqkv_h, "wo": wo_h,
             "cos_e": cs_h, "sin_e": sn_h, "consts": consts}
        if causal:
            m["dmadd"] = dmadd
        else:
            m["mask_t"] = mask_t_full
        in_maps.append(m)
    return in_maps


def _is_causal(mask2d):
    expected = np.where(
        np.tril(np.ones((L, L), dtype=bool)), np.float32(0.0), np.float32(NEG))
    return mask2d.shape == (L, L) and np.array_equal(mask2d, expected)


def kernel(x, attention_mask, cos, sin, w_qkv, w_o, _trace=False):
    from concourse.bass_utils import run_bass_kernel_spmd

    x = np.asarray(x, dtype=np.float32)
    attention_mask = np.asarray(attention_mask, dtype=np.float32)
    cos = np.asarray(cos, dtype=np.float32)
    sin = np.asarray(sin, dtype=np.float32)
    w_qkv = np.asarray(w_qkv, dtype=np.float32)
    w_o = np.asarray(w_o, dtype=np.float32)

    causal = _is_causal(attention_mask[0, 0])
    if causal not in _cache:
        _cache[causal] = _build(causal)
    nc = _cache[causal]

    in_maps = _host_inputs(x, attention_mask, cos, sin, w_qkv, w_o, causal)
    try:
        res = run_bass_kernel_spmd(nc, in_maps, list(range(NCORES)), trace=_trace)
    except Exception:
        # transient device errors (e.g. NRT_EXEC_UNIT_UNRECOVERABLE) usually
        # clear on retry
        res = run_bass_kernel_spmd(nc, in_maps, list(range(NCORES)), trace=_trace)
    out = np.zeros((L, D), np.float64)
    for c in range(NCORES):
        out += res.results[c]["out_p"].astype(np.float64)
    if _trace:
        kernel._last_exec_time_ns = res.exec_time_ns
    return out.astype(np.float32).reshape(B, L, D)
